# revision 2
# baseline (speedup 1.0000x reference)
"""CrossModalCenterLoss on 8 Trainium2 NeuronCores.

The reference masks the [B, C] distance matrix down to the label-matching
column per row BEFORE clamping, so the loss is exactly

    loss = (sum_b clip(||x_b - centers[labels_b]||^2, 1e-12, 1e12)) / B
         + (C - 1) * 1e-12

No [B, C] matmul is needed — just a gather and a fused squared-distance
reduction. Data-parallel over batch: each of the 8 cores handles 512 rows,
gathers its 512 center rows on-device via indirect DMA (centers stay in
DRAM, replicated), computes per-partition partial sums, and the host
all-reduces the 8x128 partials into the scalar loss.
"""

import numpy as np

_N_CORES = 8
_B = 4096
_D = 256
_C = 10000
_ROWS = _B // _N_CORES  # 512 rows per core
_P = 128
_K = _ROWS // _P  # 4 column blocks of 256 per partition
_CLAMP_MIN = 1e-12
_CLAMP_MAX = 1e12

_compiled = None


def _build():
    import concourse.bass as bass
    import concourse.mybir as mybir
    from concourse import bacc
    from concourse.tile import TileContext

    nc = bacc.Bacc(
        "TRN2", target_bir_lowering=False, debug=False, num_devices=_N_CORES
    )
    x = nc.declare_dram_parameter("x", [_ROWS, _D], mybir.dt.float32, isOutput=False)
    labels = nc.declare_dram_parameter(
        "labels", [_ROWS], mybir.dt.int32, isOutput=False
    )
    centers = nc.declare_dram_parameter(
        "centers", [_C, _D], mybir.dt.float32, isOutput=False
    )
    out = nc.declare_dram_parameter("out", [_P, 1], mybir.dt.float32, isOutput=True)

    with TileContext(nc) as tc:
        with tc.tile_pool(name="p", bufs=1) as pool:
            lab = pool.tile([_P, _K], mybir.dt.int32)
            xt = pool.tile([_P, _K * _D], mybir.dt.float32)
            gt = pool.tile([_P, _K * _D], mybir.dt.float32)
            diff = pool.tile([_P, _K * _D], mybir.dt.float32)
            sq = pool.tile([_P, _K * _D], mybir.dt.float32)
            part = pool.tile([_P, 1], mybir.dt.float32)

            # lab[p, k] = labels[k*128 + p]
            nc.sync.dma_start(
                out=lab[:], in_=labels[:].rearrange("(k p) -> p k", p=_P)
            )
            for k in range(_K):
                nc.sync.dma_start(
                    out=xt[:, k * _D : (k + 1) * _D],
                    in_=x[k * _P : (k + 1) * _P, :],
                )
                # gt[p, k*256:(k+1)*256] = centers[labels[k*128+p], :]
                nc.gpsimd.indirect_dma_start(
                    out=gt[:, k * _D : (k + 1) * _D],
                    out_offset=None,
                    in_=centers[:],
                    in_offset=bass.IndirectOffsetOnAxis(ap=lab[:, k : k + 1], axis=0),
                )
            nc.vector.tensor_tensor(
                out=diff[:], in0=xt[:], in1=gt[:], op=mybir.AluOpType.subtract
            )
            # part[p] = sum over free dim of diff^2  (4 rows' worth per partition)
            nc.scalar.activation(
                out=sq[:],
                in_=diff[:],
                func=mybir.ActivationFunctionType.Square,
                accum_out=part[:],
            )
            nc.sync.dma_start(out=out[:], in_=part[:])

    nc.compile()
    return nc


def _get_compiled():
    global _compiled
    if _compiled is None:
        _compiled = _build()
    return _compiled


def kernel(x, labels, centers):
    from concourse.bass_utils import run_bass_kernel_spmd

    x = np.ascontiguousarray(np.asarray(x, dtype=np.float32))
    labels_np = np.ascontiguousarray(np.asarray(labels).astype(np.int32))
    centers = np.ascontiguousarray(np.asarray(centers, dtype=np.float32))
    assert x.shape == (_B, _D) and labels_np.shape == (_B,)
    assert centers.shape == (_C, _D)

    nc = _get_compiled()
    in_maps = [
        {
            "x": np.ascontiguousarray(x[i * _ROWS : (i + 1) * _ROWS]),
            "labels": np.ascontiguousarray(labels_np[i * _ROWS : (i + 1) * _ROWS]),
            "centers": centers,
        }
        for i in range(_N_CORES)
    ]
    res = run_bass_kernel_spmd(nc, in_maps, list(range(_N_CORES)))

    # Host-side all-reduce of the per-core per-partition partials. Each row's
    # squared distance is hundreds for any non-degenerate input, so the
    # per-element clamp in the reference is a no-op on the selected entries;
    # the (C-1) masked-out zeros per row each clamp up to CLAMP_MIN.
    total = 0.0
    for i in range(_N_CORES):
        part = np.asarray(res.results[i]["out"], dtype=np.float64)
        total += float(part.sum())
    loss = total / _B + (_C - 1) * _CLAMP_MIN
    return np.asarray(loss, dtype=np.float32)


# revision 3
# speedup vs baseline: 1.1370x; 1.1370x over previous
"""CrossModalCenterLoss on 8 Trainium2 NeuronCores.

The reference masks the [B, C] distance matrix down to the label-matching
column per row BEFORE clamping, so the loss is exactly

    loss = (sum_b clip(||x_b - centers[labels_b]||^2, 1e-12, 1e12)) / B
         + (C - 1) * 1e-12

No [B, C] matmul is needed — just a gather and a fused squared-distance
reduction. Data-parallel over batch: each of the 8 cores handles 512 rows,
gathers its 512 center rows on-device via one indirect DMA (centers stay
in DRAM, replicated), computes per-partition partial sums, and the host
all-reduces the 8x128 partials into the scalar loss.

Layout: partition p of a core's tiles holds that core's rows 4p..4p+3, so
the labels DMA, the x DMA, and the gather destination are all fully
contiguous (no strided descriptors).
"""

import numpy as np

_N_CORES = 8
_B = 4096
_D = 256
_C = 10000
_ROWS = _B // _N_CORES  # 512 rows per core
_P = 128
_K = _ROWS // _P  # 4 rows per partition
_CLAMP_MIN = 1e-12
_CLAMP_MAX = 1e12

_compiled = None


def _build():
    import concourse.bass as bass
    import concourse.mybir as mybir
    from concourse import bacc
    from concourse.tile import TileContext

    nc = bacc.Bacc(
        "TRN2",
        target_bir_lowering=False,
        debug=False,
        num_devices=_N_CORES,
        enable_partition_id=False,
    )
    x = nc.declare_dram_parameter("x", [_ROWS, _D], mybir.dt.float32, isOutput=False)
    labels = nc.declare_dram_parameter(
        "labels", [_ROWS], mybir.dt.int32, isOutput=False
    )
    centers = nc.declare_dram_parameter(
        "centers", [_C, _D], mybir.dt.float32, isOutput=False
    )
    out = nc.declare_dram_parameter("out", [_P, 1], mybir.dt.float32, isOutput=True)

    F = _K * _D  # 1024 free elements per partition

    with TileContext(nc) as tc:
        with tc.tile_pool(name="p", bufs=1) as pool:
            lab = pool.tile([_P, _K], mybir.dt.int32)
            xt = pool.tile([_P, F], mybir.dt.float32)
            gt = pool.tile([_P, F], mybir.dt.float32)
            diff = pool.tile([_P, F], mybir.dt.float32)
            sq = pool.tile([_P, F], mybir.dt.float32)
            part = pool.tile([_P, 1], mybir.dt.float32)

            # lab[p, k] = labels[4p + k]  (contiguous)
            nc.sync.dma_start(
                out=lab[:], in_=labels[:].rearrange("(p k) -> p k", p=_P)
            )
            # xt[p, k*256 + d] = x[4p + k, d]  (contiguous, 4 KiB per partition)
            nc.sync.dma_start(
                out=xt[:], in_=x[:].rearrange("(p k) d -> p (k d)", p=_P)
            )
            # gt[p, k*256:(k+1)*256] = centers[lab[p, k], :] — one fused gather
            nc.gpsimd.indirect_dma_start(
                out=gt[:],
                out_offset=None,
                in_=centers[:],
                in_offset=bass.IndirectOffsetOnAxis(ap=lab[:], axis=0),
            )
            nc.vector.tensor_tensor(
                out=diff[:], in0=xt[:], in1=gt[:], op=mybir.AluOpType.subtract
            )
            nc.vector.tensor_tensor(
                out=sq[:], in0=diff[:], in1=diff[:], op=mybir.AluOpType.mult
            )
            nc.vector.reduce_sum(out=part[:], in_=sq[:], axis=mybir.AxisListType.X)
            nc.sync.dma_start(out=out[:], in_=part[:])

    nc.compile()
    return nc


def _get_compiled():
    global _compiled
    if _compiled is None:
        _compiled = _build()
    return _compiled


def kernel(x, labels, centers):
    from concourse.bass_utils import run_bass_kernel_spmd

    x = np.ascontiguousarray(np.asarray(x, dtype=np.float32))
    labels_np = np.ascontiguousarray(np.asarray(labels).astype(np.int32))
    centers = np.ascontiguousarray(np.asarray(centers, dtype=np.float32))
    assert x.shape == (_B, _D) and labels_np.shape == (_B,)
    assert centers.shape == (_C, _D)

    nc = _get_compiled()
    in_maps = [
        {
            "x": np.ascontiguousarray(x[i * _ROWS : (i + 1) * _ROWS]),
            "labels": np.ascontiguousarray(labels_np[i * _ROWS : (i + 1) * _ROWS]),
            "centers": centers,
        }
        for i in range(_N_CORES)
    ]
    res = run_bass_kernel_spmd(nc, in_maps, list(range(_N_CORES)))

    # Host-side all-reduce of the per-core per-partition partials. Each row's
    # squared distance is hundreds for any non-degenerate input, so the
    # per-element clamp in the reference is a no-op on the selected entries;
    # the (C-1) masked-out zeros per row each clamp up to CLAMP_MIN.
    total = 0.0
    for i in range(_N_CORES):
        part = np.asarray(res.results[i]["out"], dtype=np.float64)
        total += float(part.sum())
    loss = total / _B + (_C - 1) * _CLAMP_MIN
    return np.asarray(loss, dtype=np.float32)
